# revision 7
# baseline (speedup 1.0000x reference)
"""Bass/Trainium2 kernel for blockwise cross-attention (fp8 DoubleRow).

Math (per batch element, per 16-row block):
  out1 = softmax(q1 k2^T / sqrt(E)) @ v2,  out2 = softmax(q2 k1^T / sqrt(E)) @ v1
with q = x Wq^T + bq etc.  Softmax is shift-invariant along the key axis, so
the q-side bias drops:  softmax(q1 k2^T / s) == softmax(x1 A x2^T + 1 (x2 c)^T)
with A = Wq^T Wk / s and c = Wk^T bq / s precomputed on the host.

Speed comes from fp8 (e4m3) DoubleRow matmuls (0.5 cycles/row, 2x bf16) with
split-precision residual correction to keep accuracy:
  x  = x8 + r8              (two fp8 tensors, host-split)
  A^T*SA = at_hi + at_lo    (SA=2048 scales A into fp8 range)
  Wv^T*SW = W8 + rW8        (SW=64)
  z  = x8@at_hi + x8@at_lo + r8@at_hi        (3 DR products; z stored fp8 --
                                              the only uncorrected fp8 noise)
  v  = x8@W8 + r8@W8 + x8@rW8                (3 DR products -> bf16)
  sT = z8@x8^T + z8@r8^T                     (2 DR products, TRANSPOSED:
                                              k on partitions, q on free)
Transposed scores let the key-side bias factor e^{t[k]} apply as a per-
partition scalar (no [128,512] mfac tensor), and the masked exp IS already
attn^T -- no DVE transpose.  The out matmul uses attnT as stationary with
bf16 v moving; a [128,1] ones matmul gives the softmax denominator for free.
Normalization (num/den), the v bias bv, and all scale factors are applied on
the HOST after DMAing the raw f32 num straight out of PSUM (no out copy).

Sharding: pure data-parallel -- batch B=8, one batch element per NeuronCore.
"""

import math
import sys

if "/opt/trn_rl_repo" not in sys.path:
    sys.path.insert(0, "/opt/trn_rl_repo")

import numpy as np
import ml_dtypes

BF16 = ml_dtypes.bfloat16
F8 = ml_dtypes.float8_e4m3
BLOCK = 16  # attention block size (ceil(S**(2/3)) blocks => 16 for S=4096)
SA = 2048.0  # scale for A^T into fp8 range (A elems ~4e-4)
SW = 64.0  # scale for Wv^T into fp8 range (Wv elems ~0.02)


def _build_nc(S: int, E: int):
    from contextlib import ExitStack

    import concourse.bass as bass
    import concourse.tile as tile
    from concourse import bacc, mybir

    f32 = mybir.dt.float32
    bf16 = mybir.dt.bfloat16
    fp8 = mybir.dt.float8e4
    P = 128
    GROUP = 512  # rows per group
    G = S // GROUP  # 8
    NCH = E // P  # e-chunks (4)
    NW = GROUP // P  # windows per group (4)
    assert S % GROUP == 0 and E == 512

    DR = mybir.MatmulPerfMode.DoubleRow
    Exp = mybir.ActivationFunctionType.Exp

    nc = bacc.Bacc("TRN2", debug=False)

    x8_dram = [
        nc.dram_tensor("x8_1", [E, S], fp8, kind="ExternalInput").ap(),
        nc.dram_tensor("x8_2", [E, S], fp8, kind="ExternalInput").ap(),
    ]
    r8_dram = [
        nc.dram_tensor("r8_1", [E, S], fp8, kind="ExternalInput").ap(),
        nc.dram_tensor("r8_2", [E, S], fp8, kind="ExternalInput").ap(),
    ]
    athi_dram = nc.dram_tensor("at_hi", [E, E], fp8, kind="ExternalInput").ap()
    atlo_dram = nc.dram_tensor("at_lo", [E, E], fp8, kind="ExternalInput").ap()
    w8_dram = nc.dram_tensor("w8", [E, E], fp8, kind="ExternalInput").ap()
    rw8_dram = nc.dram_tensor("rw8", [E, E], fp8, kind="ExternalInput").ap()
    mask_dram = nc.dram_tensor("mask", [P, P], bf16, kind="ExternalInput").ap()
    ones_dram = nc.dram_tensor("ones", [P, 1], bf16, kind="ExternalInput").ap()
    # etp[p, s*32+wg] = exp(t_s[wg*128+p]) -- key-side bias factor per row
    et_dram = nc.dram_tensor("etp", [P, 2 * (S // P)], f32, kind="ExternalInput").ap()
    num_dram = [
        nc.dram_tensor("num1", [S, E], bf16, kind="ExternalOutput").ap(),
        nc.dram_tensor("num2", [S, E], bf16, kind="ExternalOutput").ap(),
    ]
    # den[g, q, qs*NW+w] = softmax denominator for out{qs+1} row g*512+w*128+q
    den_dram = nc.dram_tensor("den", [G, P, 2 * NW], f32, kind="ExternalOutput").ap()

    with ExitStack() as ctx:
        tc = ctx.enter_context(tile.TileContext(nc))

        consts = ctx.enter_context(tc.tile_pool(name="consts", bufs=1))
        x_pool = ctx.enter_context(tc.tile_pool(name="xp", bufs=2))
        z_pool = ctx.enter_context(tc.tile_pool(name="zp", bufs=2))
        v_pool = ctx.enter_context(tc.tile_pool(name="vp", bufs=2))
        sm_pool = ctx.enter_context(tc.tile_pool(name="sm", bufs=4))
        psZV = ctx.enter_context(tc.tile_pool(name="psZV", bufs=3, space="PSUM"))
        psS = ctx.enter_context(tc.tile_pool(name="psS", bufs=2, space="PSUM"))
        psO = ctx.enter_context(tc.tile_pool(name="psO", bufs=2, space="PSUM"))
        psD = ctx.enter_context(tc.tile_pool(name="psD", bufs=1, space="PSUM"))

        # --- constants ---
        def load_w(name, dram):
            t = consts.tile([P, NCH * E], fp8, name=name, tag=name)
            nc.sync.dma_start(
                t.rearrange("p (c e) -> p c e", c=NCH),
                dram.rearrange("(c p) e -> p c e", p=P),
            )
            return t

        at_hi = load_w("athi", athi_dram)
        at_lo = load_w("atlo", atlo_dram)
        w8_t = load_w("w8t", w8_dram)
        rw8_t = load_w("rw8t", rw8_dram)
        mask_t = consts.tile([P, P], bf16, name="mask", tag="mask")
        nc.scalar.dma_start(mask_t[:], mask_dram[:])
        ones_t = consts.tile([P, 1], bf16, name="ones", tag="ones")
        nc.scalar.dma_start(ones_t[:], ones_dram[:])
        et_t = consts.tile([P, 2 * (S // P)], f32, name="etp", tag="etp")
        nc.scalar.dma_start(et_t[:], et_dram[:])

        def wpair(t, cp):  # [128, 2, E] c-pair view of a weight tile
            return t.rearrange("p (c e) -> p c e", c=NCH)[:, cp : cp + 2, :]

        def wpair_m(t, cp, m):  # [128, 2, 128] c-pair view, e_out m-slice
            return t.rearrange("p (c e) -> p c e", c=NCH)[
                :, cp : cp + 2, m * P : (m + 1) * P
            ]

        st = {}
        cp_rr = [0]

        def copy_rr(out_ap, in_ap):
            e = cp_rr[0] % 2
            cp_rr[0] += 1
            if e == 0:
                nc.scalar.copy(out_ap, in_ap)
            else:
                nc.vector.tensor_copy(out_ap, in_ap)

        def emit_load_proj(g):
            r0 = g * GROUP
            x8 = {}
            r8 = {}
            zt = {}
            vt = {}
            for s in range(2):
                for nm, dram, d in (("x8", x8_dram, x8), ("r8", r8_dram, r8)):
                    tl = x_pool.tile(
                        [P, NCH * GROUP], fp8, name=f"{nm}{s}", tag=f"{nm}{s}"
                    )
                    nc.sync.dma_start(
                        tl.rearrange("p (c r) -> p c r", c=NCH),
                        dram[s].rearrange("(c p) s -> p c s", p=P)[
                            :, :, r0 : r0 + GROUP
                        ],
                    )
                    d[s] = tl

            def xpair(d, s, cp, rs=None):  # [128, 2, *] c-pair view of x/r tile
                v = d[s].rearrange("p (c r) -> p c r", c=NCH)
                return v[:, cp : cp + 2, :] if rs is None else v[:, cp : cp + 2, rs]

            for s in range(2):
                # z^T m-chunks [128 e_out, 512 rows], fp8 (3 DR products)
                z_tl = z_pool.tile([P, NCH * GROUP], fp8, name=f"zt{s}", tag=f"zt{s}")
                for m in range(NCH):
                    z_ps = psZV.tile([P, GROUP], f32, name="zps", tag="psZV")
                    mms = [(at_hi, x8), (at_lo, x8), (at_hi, r8)]
                    n = 0
                    for wt, xd in mms:
                        for cp in (0, 2):
                            nc.tensor.matmul(
                                z_ps[:], wpair_m(wt, cp, m), xpair(xd, s, cp),
                                start=(n == 0), stop=(n == 5), perf_mode=DR,
                            )
                            n += 1
                    copy_rr(z_tl[:, m * GROUP : (m + 1) * GROUP], z_ps[:])
                zt[s] = z_tl

                # v' r-chunks [128 rows, E] = SW * x Wv^T, bf16 (3 DR products)
                for rc in range(NW):
                    v_ps = psZV.tile([P, E], f32, name="vps", tag="psZV")
                    rs = slice(rc * P, (rc + 1) * P)
                    mms = [(x8, w8_t), (r8, w8_t), (x8, rw8_t)]
                    n = 0
                    for xd, wt in mms:
                        for cp in (0, 2):
                            nc.tensor.matmul(
                                v_ps[:], xpair(xd, s, cp, rs), wpair(wt, cp),
                                start=(n == 0), stop=(n == 5), perf_mode=DR,
                            )
                            n += 1
                    v_sb = v_pool.tile([P, E], bf16, name=f"vsb{s}{rc}", tag=f"vsb{s}{rc}")
                    copy_rr(v_sb[:], v_ps[:])
                    vt[s, rc] = v_sb
            st[g] = (x8, r8, zt, vt)

        def emit_attn(g):
            r0 = g * GROUP
            x8, r8, zt, vt = st.pop(g)
            den_ps = psD.tile([P, 2 * NW], f32, name="dps", tag="psD")
            for w in range(NW):
                ws = slice(w * P, (w + 1) * P)
                for qs, ks in ((0, 1), (1, 0)):
                    # sT[k, q] (transposed scores): 2 DR products over m-pairs
                    s_ps = psS.tile([P, P], f32, name="sps", tag="psS")
                    zv = zt[ks].rearrange("p (m r) -> p m r", m=NCH)
                    n = 0
                    for xd in (x8, r8):
                        xv = xd[qs].rearrange("p (c r) -> p c r", c=NCH)
                        for mp in (0, 2):
                            nc.tensor.matmul(
                                s_ps[:], zv[:, mp : mp + 2, ws], xv[:, mp : mp + 2, ws],
                                start=(n == 0), stop=(n == 3), perf_mode=DR,
                            )
                            n += 1
                    # exp(sT/SA) -> bf16 (ACT)
                    exp_sb = sm_pool.tile([P, P], bf16, name="expsb", tag="expsb")
                    nc.scalar.activation(exp_sb[:], s_ps[:], Exp, scale=1.0 / SA)
                    # masked attn^T = exp * e^{t[k]} * mask[k,q]  (DVE)
                    attnT = sm_pool.tile([P, P], bf16, name="attnT", tag="attnT")
                    nc.vector.scalar_tensor_tensor(
                        attnT[:], exp_sb[:],
                        et_t[:, ks * (S // P) + g * NW + w : ks * (S // P) + g * NW + w + 1],
                        mask_t[:],
                        op0=mybir.AluOpType.mult, op1=mybir.AluOpType.mult,
                    )
                    # denominator: attnT^T @ ones -> [128 q, 1]
                    nc.tensor.matmul(
                        den_ps[:, qs * NW + w : qs * NW + w + 1],
                        attnT[:], ones_t[:], start=True, stop=True,
                    )
                    # numerator: attnT^T @ v -> [128 q, E] f32 -> bf16 -> DMA
                    o_ps = psO.tile([P, E], f32, name="ops", tag="psO")
                    nc.tensor.matmul(o_ps[:], attnT[:], vt[ks, w][:], start=True, stop=True)
                    o_sb = sm_pool.tile([P, E], bf16, name="osb", tag="osb")
                    copy_rr(o_sb[:], o_ps[:])
                    nc.sync.dma_start(
                        num_dram[qs][r0 + w * P : r0 + (w + 1) * P, :], o_sb[:]
                    )
            den_sb = sm_pool.tile([P, 2 * NW], f32, name="dsb", tag="dsb")
            nc.vector.tensor_copy(den_sb[:], den_ps[:])
            nc.sync.dma_start(den_dram[g], den_sb[:])

        for g in range(G):
            emit_load_proj(g)
            emit_attn(g)

    nc.compile()
    return nc


def _host_inputs(state1, state2, Wq, bq, Wk, bk, Wv, bv, S, E):
    """Build per-core input maps (weights common, x per-core)."""
    P = 128
    scale = math.sqrt(E)
    Wq64 = np.asarray(Wq, np.float64)
    Wk64 = np.asarray(Wk, np.float64)
    atm = (Wk64.T @ Wq64 / scale).astype(np.float32)  # A^T [e_in, e_out]
    cvec = (Wk64.T @ np.asarray(bq, np.float64) / scale).astype(np.float32)

    def split8(a):
        hi = a.astype(F8)
        lo = (a - hi.astype(np.float32)).astype(F8)
        return np.ascontiguousarray(hi), np.ascontiguousarray(lo)

    at_hi, at_lo = split8(atm * SA)
    w8, rw8 = split8(np.asarray(Wv, np.float32).T * SW)
    idx = np.arange(P)
    mask = (idx[:, None] // BLOCK == idx[None, :] // BLOCK).astype(BF16)
    ones = np.ones((P, 1), BF16)
    common = {
        "at_hi": at_hi, "at_lo": at_lo, "w8": w8, "rw8": rw8,
        "mask": np.ascontiguousarray(mask), "ones": ones,
    }
    x1 = np.asarray(state1, np.float32)
    x2 = np.asarray(state2, np.float32)
    B = x1.shape[0]
    NWIN = S // P
    per_core = []
    for b in range(B):
        m = dict(common)
        etp = np.empty((P, 2 * NWIN), np.float32)
        for s, x in ((0, x1[b]), (1, x2[b])):
            xt = np.ascontiguousarray(x.T)  # [E, S]
            x8 = xt.astype(F8)
            r8 = (xt - x8.astype(np.float32)).astype(F8)
            m[f"x8_{s + 1}"] = x8
            m[f"r8_{s + 1}"] = r8
            # et[p, wg] = exp(t[wg*128+p])
            t = x @ cvec
            etp[:, s * NWIN : (s + 1) * NWIN] = np.exp(t).reshape(NWIN, P).T
        m["etp"] = etp
        per_core.append(m)
    return per_core


_NC_CACHE = {}


def _get_nc(S, E):
    key = (S, E)
    if key not in _NC_CACHE:
        _NC_CACHE[key] = _build_nc(S, E)
    return _NC_CACHE[key]


def kernel(state1, state2, Wq, bq, Wk, bk, Wv, bv):
    from concourse.bass_utils import run_bass_kernel_spmd

    state1 = np.asarray(state1)
    B, S, E = state1.shape
    assert (B, S, E) == (8, 4096, 512), (B, S, E)
    P, GROUP = 128, 512
    G, NW = S // GROUP, GROUP // P

    nc = _get_nc(S, E)
    in_maps = _host_inputs(state1, state2, Wq, bq, Wk, bk, Wv, bv, S, E)
    res = run_bass_kernel_spmd(nc, in_maps, list(range(B)))
    bvf = np.asarray(bv, np.float32)[None, :]
    outs = [np.empty((B, S, E), np.float32) for _ in range(2)]
    for b in range(B):
        den = np.asarray(res.results[b]["den"], np.float32)  # [G, P, 2*NW]
        for qs in range(2):
            num = np.asarray(res.results[b][f"num{qs + 1}"]).astype(np.float32)
            d = den[:, :, qs * NW : (qs + 1) * NW]  # [G, P, NW]
            d = d.transpose(0, 2, 1).reshape(S, 1)  # row-major denominators
            outs[qs][b] = num / (d * SW) + bvf
    return outs[0], outs[1]


if __name__ == "__main__":
    rng = np.random.default_rng(0)
    B, S, E = 8, 4096, 512
    ins = {
        "state1": rng.standard_normal((B, S, E), np.float32),
        "state2": rng.standard_normal((B, S, E), np.float32),
        "Wq": rng.standard_normal((E, E), np.float32) * 0.02,
        "bq": rng.standard_normal((E,), np.float32) * 0.02,
        "Wk": rng.standard_normal((E, E), np.float32) * 0.02,
        "bk": rng.standard_normal((E,), np.float32) * 0.02,
        "Wv": rng.standard_normal((E, E), np.float32) * 0.02,
        "bv": rng.standard_normal((E,), np.float32) * 0.02,
    }
    o1, o2 = kernel(**ins)
    print("ok", o1.shape, o2.shape, o1.dtype)


# revision 8
# speedup vs baseline: 1.2973x; 1.2973x over previous
"""Bass/Trainium2 kernel for blockwise cross-attention.

Math (per batch element b, per 16-row block):
  out1 = softmax(q1 k2^T / sqrt(E)) @ v2,  out2 = softmax(q2 k1^T / sqrt(E)) @ v1
with q = x Wq^T + bq etc.  Since softmax is shift-invariant along the key
axis, the q-side bias terms drop and
  softmax(q1 k2^T / s) == softmax(x1 A x2^T + 1 (x2 c)^T)
with A = Wq^T Wk / s and c = Wk^T bq / s precomputed on the host.  This
replaces 6 big projections with 4 (z = x A^T fused for both q&k roles, plus
v' = x Wv^T).  The v bias folds in exactly because softmax rows sum to 1.

The z projection runs in fp8 (e4m3) DoubleRow mode: A^T*SA is host-split
into at_hi + at_lo (residual-corrected, so of the fp8 quantizations only the
x8 noise remains — and that is softmax-damped), and each DR matmul contracts
two 128-chunks at once at 0.5 cycles/row.  This halves both the matmul and
the LDWEIGHTS cost of the z GEMMs vs bf16.  v stays bf16 (its error path
reaches the output unsoftened; fp8 there would breach the error budget).

Sharding: pure data-parallel — batch B=8, one batch element per NeuronCore.

Device flow per core (S=4096 rows, E=512), fp32 softmax:
  - x^T tiles bf16 [128e, 512rows] (scores/v stationaries) + fp8 copy (z)
  - z^T = (at_hi + at_lo) @ x8 via DR c-pairs; v' = x Wv^T natural bf16
  - scores window [128q,128k]: 4 e-chunk bf16 matmuls (stationary x^T bf16,
    moving z^T bf16); exp via ACT with scale=1/SA (z carries the SA scale)
  - softmax: DVE (exp*mfac) with fused accum row-sum (mfac = blockmask *
    e^{t[k]} host-precomputed f32), reciprocal, 32x32-block transpose ==
    exact transpose of the block-diagonal attn
  - out = attnT.T @ v' single K=128 matmul -> PSUM -> stt(rcp, +bv) -> DMA
"""

import math
import sys

if "/opt/trn_rl_repo" not in sys.path:
    sys.path.insert(0, "/opt/trn_rl_repo")

import numpy as np
import ml_dtypes

BF16 = ml_dtypes.bfloat16
F8 = ml_dtypes.float8_e4m3
BLOCK = 16  # attention block size (ceil(S**(2/3)) blocks => 16 for S=4096)
SA = 2048.0  # scale for A^T into fp8 range (A elems ~4e-4)


def _build_nc(S: int, E: int):
    from contextlib import ExitStack

    import concourse.bass as bass
    import concourse.tile as tile
    from concourse import bacc, mybir

    f32 = mybir.dt.float32
    bf16 = mybir.dt.bfloat16
    fp8 = mybir.dt.float8e4
    P = 128
    GROUP = 512  # rows per group
    G = S // GROUP
    NCH = E // P  # e-chunks (4)
    NW = GROUP // P  # windows per group (4)
    assert S % GROUP == 0 and E == 512

    DR = mybir.MatmulPerfMode.DoubleRow
    Exp = mybir.ActivationFunctionType.Exp

    nc = bacc.Bacc("TRN2", debug=False)

    x_dram = [
        nc.dram_tensor("x1t", [E, S], bf16, kind="ExternalInput").ap(),
        nc.dram_tensor("x2t", [E, S], bf16, kind="ExternalInput").ap(),
    ]
    x8_dram = [
        nc.dram_tensor("x8_1", [E, S], fp8, kind="ExternalInput").ap(),
        nc.dram_tensor("x8_2", [E, S], fp8, kind="ExternalInput").ap(),
    ]
    athi_dram = nc.dram_tensor("at_hi", [E, E], fp8, kind="ExternalInput").ap()
    atlo_dram = nc.dram_tensor("at_lo", [E, E], fp8, kind="ExternalInput").ap()
    wvt_dram = nc.dram_tensor("wvt", [E, E], bf16, kind="ExternalInput").ap()
    # per-(state, group) post-exp factor M[q,k] = e^{t[k]} * [q,k same block]
    mf_dram = nc.dram_tensor("mfac", [2, G, P, GROUP], f32, kind="ExternalInput").ap()
    bvb_dram = nc.dram_tensor("bvb", [P, E], f32, kind="ExternalInput").ap()
    out_dram = [
        nc.dram_tensor("out1", [S, E], f32, kind="ExternalOutput").ap(),
        nc.dram_tensor("out2", [S, E], f32, kind="ExternalOutput").ap(),
    ]

    with ExitStack() as ctx:
        tc = ctx.enter_context(tile.TileContext(nc))

        consts = ctx.enter_context(tc.tile_pool(name="consts", bufs=1))
        xt_pool = ctx.enter_context(tc.tile_pool(name="xt", bufs=2))
        x8_pool = ctx.enter_context(tc.tile_pool(name="x8", bufs=2))
        z_pool = ctx.enter_context(tc.tile_pool(name="z", bufs=2))
        v_pool = ctx.enter_context(tc.tile_pool(name="v", bufs=2))
        mf_pool = ctx.enter_context(tc.tile_pool(name="mf", bufs=2))
        sm_pool = ctx.enter_context(tc.tile_pool(name="sm", bufs=3))
        o_pool = ctx.enter_context(tc.tile_pool(name="o", bufs=3))
        psA = ctx.enter_context(tc.tile_pool(name="psA", bufs=3, space="PSUM"))
        psS = ctx.enter_context(tc.tile_pool(name="psS", bufs=3, space="PSUM"))
        psO = ctx.enter_context(tc.tile_pool(name="psO", bufs=2, space="PSUM"))

        # --- constants ---
        at_hi = consts.tile([P, NCH * E], fp8, name="athi", tag="athi")
        nc.sync.dma_start(
            at_hi.rearrange("p (c e) -> p c e", c=NCH),
            athi_dram.rearrange("(c p) e -> p c e", p=P),
        )
        at_lo = consts.tile([P, NCH * E], fp8, name="atlo", tag="atlo")
        nc.sync.dma_start(
            at_lo.rearrange("p (c e) -> p c e", c=NCH),
            atlo_dram.rearrange("(c p) e -> p c e", p=P),
        )
        wv_t = consts.tile([P, NCH * E], bf16, name="wvt", tag="wvt")
        nc.scalar.dma_start(
            wv_t.rearrange("p (c e) -> p c e", c=NCH),
            wvt_dram.rearrange("(c p) e -> p c e", p=P),
        )
        bvb_t = consts.tile([P, E], f32, name="bvb", tag="bvb")
        nc.scalar.dma_start(bvb_t[:], bvb_dram[:])

        def at_pair(t, cp, m):  # [128, 2, 128] c-pair view, e_out m-slice
            return t.rearrange("p (c e) -> p c e", c=NCH)[
                :, cp : cp + 2, m * P : (m + 1) * P
            ]

        def wv_c(c):
            return wv_t[:, c * E : (c + 1) * E]

        st = {}  # per-group state: (xt, zt, vt, mf)

        def emit_load_proj(g):
            r0 = g * GROUP
            xt = {}
            x8 = {}
            zt = {}
            vt = {}
            mf = {}
            for s in range(2):
                x_tl = xt_pool.tile([P, NCH * GROUP], bf16, name=f"xt{s}", tag=f"xt{s}")
                nc.sync.dma_start(
                    x_tl.rearrange("p (c r) -> p c r", c=NCH),
                    x_dram[s].rearrange("(c p) s -> p c s", p=P)[:, :, r0 : r0 + GROUP],
                )
                xt[s] = x_tl
                x8_tl = x8_pool.tile([P, NCH * GROUP], fp8, name=f"x8{s}", tag=f"x8{s}")
                nc.sync.dma_start(
                    x8_tl.rearrange("p (c r) -> p c r", c=NCH),
                    x8_dram[s].rearrange("(c p) s -> p c s", p=P)[:, :, r0 : r0 + GROUP],
                )
                x8[s] = x8_tl

            def xt_c(s, c):  # x^T chunk c: [128 e_in, 512 rows] bf16
                return xt[s][:, c * GROUP : (c + 1) * GROUP]

            def x8_pair(s, cp):  # [128, 2, 512] fp8 c-pair view
                return x8[s].rearrange("p (c r) -> p c r", c=NCH)[:, cp : cp + 2, :]

            for s in range(2):
                # z_s^T m-chunk [128 e_out, GROUP rows] = SA * z, fp8 DR pairs
                for m in range(NCH):
                    z_ps = psA.tile([P, GROUP], f32, name="zps", tag="psA")
                    n = 0
                    for at_t in (at_hi, at_lo):
                        for cp in (0, 2):
                            nc.tensor.matmul(
                                z_ps[:], at_pair(at_t, cp, m), x8_pair(s, cp),
                                start=(n == 0), stop=(n == 3), perf_mode=DR,
                            )
                            n += 1
                    z_sb = z_pool.tile([P, GROUP], bf16, name=f"zsb{s}{m}", tag=f"zsb{s}{m}")
                    nc.scalar.copy(z_sb[:], z_ps[:])
                    zt[s, m] = z_sb

                # v'_s r-chunk [128 rows, E] = x @ Wv^T (bv added on out-copy)
                for r in range(NW):
                    v_ps = psA.tile([P, E], f32, name="vps", tag="psA")
                    for c in range(NCH):
                        nc.tensor.matmul(
                            v_ps[:], xt_c(s, c)[:, r * P : (r + 1) * P], wv_c(c),
                            start=(c == 0), stop=(c == NCH - 1),
                        )
                    v_sb = v_pool.tile([P, E], bf16, name=f"vsb{s}{r}", tag=f"vsb{s}{r}")
                    nc.scalar.copy(v_sb[:], v_ps[:])
                    vt[s, r] = v_sb

            # post-exp factor tiles — emitted after the projections so these
            # loads don't compete with the critical x/at transfers
            for s in range(2):
                mf_tl = mf_pool.tile([P, GROUP], f32, name=f"mf{s}", tag=f"mf{s}")
                nc.sync.dma_start(mf_tl[:], mf_dram[s, g])
                mf[s] = mf_tl
            st[g] = (xt, zt, vt, mf)

        def emit_attn(g):
            r0 = g * GROUP
            xt, zt, vt, mf = st.pop(g)
            for w in range(NW):
                ws = slice(w * P, (w + 1) * P)
                for qs, ks in ((0, 1), (1, 0)):
                    s_ps = psS.tile([P, P], f32, name="sps", tag="psS")
                    for m in range(NCH):
                        nc.tensor.matmul(
                            s_ps[:],
                            xt[qs][:, m * GROUP + w * P : m * GROUP + (w + 1) * P],
                            zt[ks, m][:, ws],
                            start=(m == 0), stop=(m == NCH - 1),
                        )
                    exp_sb = sm_pool.tile([P, P], f32, name="expsb", tag="expsb")
                    nc.scalar.activation(exp_sb[:], s_ps[:], Exp, scale=1.0 / SA)
                    # masked UNNORMALIZED attn = exp * M (zeroes off-block,
                    # applies e^{t[k]}), fused row-sum in the same DVE op;
                    # normalization happens per-q-row on the out-copy below
                    mskd = sm_pool.tile([P, P], bf16, name="mskd", tag="mskd")
                    rsum = sm_pool.tile([P, 1], f32, name="rsum", tag="rsum")
                    nc.vector.scalar_tensor_tensor(
                        mskd[:], exp_sb[:], 1.0, mf[ks][:, ws],
                        op0=mybir.AluOpType.mult, op1=mybir.AluOpType.mult,
                        accum_out=rsum[:],
                    )
                    rcp = sm_pool.tile([P, 1], f32, name="rcp", tag="rcp")
                    nc.vector.reciprocal(rcp[:], rsum[:])
                    attnT = sm_pool.tile([P, P], bf16, name="attnT", tag="attnT")
                    nc.vector.transpose(attnT[:], mskd[:])

                    o_ps = psO.tile([P, E], f32, name="ops", tag="psO")
                    nc.tensor.matmul(o_ps[:], attnT[:], vt[ks, w][:], start=True, stop=True)
                    # out = (attn_unnorm @ v) * recip[q] + bv  — one DVE op
                    o_sb = o_pool.tile([P, E], f32, name=f"osb{qs}", tag=f"osb{qs}")
                    nc.vector.scalar_tensor_tensor(
                        o_sb[:], o_ps[:], rcp[:], bvb_t[:],
                        op0=mybir.AluOpType.mult, op1=mybir.AluOpType.add,
                    )
                    nc.gpsimd.dma_start(out_dram[qs][r0 + w * P : r0 + (w + 1) * P, :], o_sb[:])

        for g in range(G):
            emit_load_proj(g)
            emit_attn(g)

    nc.compile()
    return nc


def _host_inputs(state1, state2, Wq, bq, Wk, bk, Wv, bv, S, E):
    """Build the per-core common (weight) arrays + per-core x arrays."""
    P = 128
    GROUP = 512
    G = S // GROUP
    scale = math.sqrt(E)
    Wq64 = np.asarray(Wq, np.float64)
    Wk64 = np.asarray(Wk, np.float64)
    # A = Wq^T Wk / scale ; device needs A^T = Wk^T Wq / scale  [e_in, e_out]
    atm = (Wk64.T @ Wq64 / scale).astype(np.float32)
    at_hi = (atm * SA).astype(F8)
    at_lo = (atm * SA - at_hi.astype(np.float32)).astype(F8)
    cvec = (Wk64.T @ np.asarray(bq, np.float64) / scale).astype(np.float32)  # [E]
    wvt = np.ascontiguousarray(np.asarray(Wv, np.float32).T).astype(BF16)
    bvb = np.broadcast_to(np.asarray(bv, np.float32).reshape(1, E), (P, E))
    common = {
        "at_hi": np.ascontiguousarray(at_hi),
        "at_lo": np.ascontiguousarray(at_lo),
        "wvt": wvt,
        "bvb": np.ascontiguousarray(bvb),
    }
    # post-exp factor M[q, k] = [q, k in same 16-block] * e^{t[k]}
    idx = np.arange(P)
    kidx = np.arange(GROUP) % P
    pattern = (idx[:, None] // BLOCK == kidx[None, :] // BLOCK).astype(np.float32)
    x1 = np.asarray(state1, np.float32)
    x2 = np.asarray(state2, np.float32)
    B = x1.shape[0]
    per_core = []
    for b in range(B):
        mfac = np.empty((2, G, P, GROUP), np.float32)
        for s, x in ((0, x1[b]), (1, x2[b])):
            et = np.exp(x @ cvec).reshape(G, 1, GROUP)
            mfac[s] = pattern[None, :, :] * et
        x1t = np.ascontiguousarray(x1[b].T)
        x2t = np.ascontiguousarray(x2[b].T)
        per_core.append(
            {
                "x1t": x1t.astype(BF16),
                "x2t": x2t.astype(BF16),
                "x8_1": x1t.astype(F8),
                "x8_2": x2t.astype(F8),
                "mfac": mfac,
                **common,
            }
        )
    return per_core


_NC_CACHE = {}


def _get_nc(S, E):
    key = (S, E)
    if key not in _NC_CACHE:
        _NC_CACHE[key] = _build_nc(S, E)
    return _NC_CACHE[key]


def kernel(state1, state2, Wq, bq, Wk, bk, Wv, bv):
    from concourse.bass_utils import run_bass_kernel_spmd

    state1 = np.asarray(state1)
    B, S, E = state1.shape
    assert (B, S, E) == (8, 4096, 512), (B, S, E)

    nc = _get_nc(S, E)
    in_maps = _host_inputs(state1, state2, Wq, bq, Wk, bk, Wv, bv, S, E)
    res = run_bass_kernel_spmd(nc, in_maps, list(range(B)))
    out1 = np.stack([res.results[b]["out1"] for b in range(B)])
    out2 = np.stack([res.results[b]["out2"] for b in range(B)])
    return out1, out2


if __name__ == "__main__":
    rng = np.random.default_rng(0)
    B, S, E = 8, 4096, 512
    ins = {
        "state1": rng.standard_normal((B, S, E), np.float32),
        "state2": rng.standard_normal((B, S, E), np.float32),
        "Wq": rng.standard_normal((E, E), np.float32) * 0.02,
        "bq": rng.standard_normal((E,), np.float32) * 0.02,
        "Wk": rng.standard_normal((E, E), np.float32) * 0.02,
        "bk": rng.standard_normal((E,), np.float32) * 0.02,
        "Wv": rng.standard_normal((E, E), np.float32) * 0.02,
        "bv": rng.standard_normal((E,), np.float32) * 0.02,
    }
    o1, o2 = kernel(**ins)
    print("ok", o1.shape, o2.shape, o1.dtype)


# revision 9
# speedup vs baseline: 1.4403x; 1.1102x over previous
"""Bass/Trainium2 kernel for blockwise cross-attention.

Math (per batch element b, per 16-row block):
  out1 = softmax(q1 k2^T / sqrt(E)) @ v2,  out2 = softmax(q2 k1^T / sqrt(E)) @ v1
with q = x Wq^T + bq etc.  Since softmax is shift-invariant along the key
axis, the q-side bias terms drop and
  softmax(q1 k2^T / s) == softmax(x1 A x2^T + 1 (x2 c)^T)
with A = Wq^T Wk / s and c = Wk^T bq / s precomputed on the host.  This
replaces 6 big projections with 4 (z = x A^T fused for both q&k roles, plus
v' = x Wv^T).  The v bias folds in exactly because softmax rows sum to 1.

The z projection runs in fp8 (e4m3) DoubleRow mode: each DR matmul contracts
two 128-chunks at once, halving the z instruction count vs bf16 (measured:
a DR matmul with N=512 moving costs the same ~217ns as a bf16 one but does
2x the contraction).  The fp8 noise this injects (A and x quantization) is
softmax-damped and fits the error budget.  v stays bf16 — its error path
reaches the output unsoftened, fp8 there would breach the budget.  Scores
and out matmuls are bf16.

Emission order keeps the PE busy: for each group, the score matmuls are
emitted first, then the NEXT group's projections, then this group's out
matmuls — so the PE chews on projections while ACT/DVE run the softmax
chains (exp -> mask-mult -> reciprocal -> transpose), instead of stalling
on them.  Exp and the 32-block transpose are batched over direction pairs
([128,256] tiles) to halve their fixed per-op overhead.

Sharding: pure data-parallel — batch B=8, one batch element per NeuronCore.
"""

import math
import sys

if "/opt/trn_rl_repo" not in sys.path:
    sys.path.insert(0, "/opt/trn_rl_repo")

import numpy as np
import ml_dtypes

BF16 = ml_dtypes.bfloat16
F8 = ml_dtypes.float8_e4m3
BLOCK = 16  # attention block size (ceil(S**(2/3)) blocks => 16 for S=4096)
SA = 2048.0  # scale for A^T into fp8 range (A elems ~4e-4)


def _build_nc(S: int, E: int):
    from contextlib import ExitStack

    import concourse.bass as bass
    import concourse.tile as tile
    from concourse import bacc, mybir

    f32 = mybir.dt.float32
    bf16 = mybir.dt.bfloat16
    fp8 = mybir.dt.float8e4
    P = 128
    GROUP = 512  # rows per group
    G = S // GROUP
    NCH = E // P  # e-chunks (4)
    NW = GROUP // P  # windows per group (4)
    assert S % GROUP == 0 and E == 512

    DR = mybir.MatmulPerfMode.DoubleRow
    Exp = mybir.ActivationFunctionType.Exp

    nc = bacc.Bacc("TRN2", debug=False)

    x_dram = [
        nc.dram_tensor("x1t", [E, S], bf16, kind="ExternalInput").ap(),
        nc.dram_tensor("x2t", [E, S], bf16, kind="ExternalInput").ap(),
    ]
    x8_dram = [
        nc.dram_tensor("x8_1", [E, S], fp8, kind="ExternalInput").ap(),
        nc.dram_tensor("x8_2", [E, S], fp8, kind="ExternalInput").ap(),
    ]
    athi_dram = nc.dram_tensor("at_hi", [E, E], fp8, kind="ExternalInput").ap()
    wvt_dram = nc.dram_tensor("wvt", [E, E], bf16, kind="ExternalInput").ap()
    # per-(state, group) post-exp factor M[q,k] = e^{t[k]} * [q,k same block]
    mf_dram = nc.dram_tensor("mfac", [2, G, P, GROUP], bf16, kind="ExternalInput").ap()
    bvb_dram = nc.dram_tensor("bvb", [P, E], f32, kind="ExternalInput").ap()
    out_dram = [
        nc.dram_tensor("out1", [S, E], f32, kind="ExternalOutput").ap(),
        nc.dram_tensor("out2", [S, E], f32, kind="ExternalOutput").ap(),
    ]

    with ExitStack() as ctx:
        tc = ctx.enter_context(tile.TileContext(nc))

        consts = ctx.enter_context(tc.tile_pool(name="consts", bufs=1))
        xt_pool = ctx.enter_context(tc.tile_pool(name="xt", bufs=2))
        x8_pool = ctx.enter_context(tc.tile_pool(name="x8", bufs=2))
        z_pool = ctx.enter_context(tc.tile_pool(name="z", bufs=2))
        v_pool = ctx.enter_context(tc.tile_pool(name="v", bufs=2))
        mf_pool = ctx.enter_context(tc.tile_pool(name="mf", bufs=2))
        sm_pool = ctx.enter_context(tc.tile_pool(name="sm", bufs=3))
        an_pool = ctx.enter_context(tc.tile_pool(name="an", bufs=8))
        o_pool = ctx.enter_context(tc.tile_pool(name="o", bufs=3))
        psA = ctx.enter_context(tc.tile_pool(name="psA", bufs=3, space="PSUM"))
        psS = ctx.enter_context(tc.tile_pool(name="psS", bufs=3, space="PSUM"))
        psO = ctx.enter_context(tc.tile_pool(name="psO", bufs=2, space="PSUM"))

        # --- constants (at_hi first: the first z matmul needs it) ---
        at_hi = consts.tile([P, NCH * E], fp8, name="athi", tag="athi")
        nc.sync.dma_start(
            at_hi.rearrange("p (c e) -> p c e", c=NCH),
            athi_dram.rearrange("(c p) e -> p c e", p=P),
        )
        wv_t = consts.tile([P, NCH * E], bf16, name="wvt", tag="wvt")
        nc.scalar.dma_start(
            wv_t.rearrange("p (c e) -> p c e", c=NCH),
            wvt_dram.rearrange("(c p) e -> p c e", p=P),
        )
        bvb_t = consts.tile([P, E], f32, name="bvb", tag="bvb")
        nc.scalar.dma_start(bvb_t[:], bvb_dram[:])

        def at_pair(cp, m):  # [128, 2, 128] c-pair view, e_out m-slice
            return at_hi.rearrange("p (c e) -> p c e", c=NCH)[
                :, cp : cp + 2, m * P : (m + 1) * P
            ]

        def wv_c(c):
            return wv_t[:, c * E : (c + 1) * E]

        st = {}  # per-group: (xt, zt, vt, mf)
        sm = {}  # per-group: list of (attnT, rcp) per window

        def emit_load_proj(g):
            r0 = g * GROUP
            xt = {}
            x8 = {}
            zt = {}
            vt = {}
            mf = {}
            for s in range(2):
                x8_tl = x8_pool.tile([P, NCH * GROUP], fp8, name=f"x8{s}", tag=f"x8{s}")
                nc.sync.dma_start(
                    x8_tl.rearrange("p (c r) -> p c r", c=NCH),
                    x8_dram[s].rearrange("(c p) s -> p c s", p=P)[:, :, r0 : r0 + GROUP],
                )
                x8[s] = x8_tl
                x_tl = xt_pool.tile([P, NCH * GROUP], bf16, name=f"xt{s}", tag=f"xt{s}")
                nc.sync.dma_start(
                    x_tl.rearrange("p (c r) -> p c r", c=NCH),
                    x_dram[s].rearrange("(c p) s -> p c s", p=P)[:, :, r0 : r0 + GROUP],
                )
                xt[s] = x_tl

            def xt_c(s, c):  # x^T chunk c: [128 e_in, 512 rows] bf16
                return xt[s][:, c * GROUP : (c + 1) * GROUP]

            def x8_pair(s, cp):  # [128, 2, 512] fp8 c-pair view
                return x8[s].rearrange("p (c r) -> p c r", c=NCH)[:, cp : cp + 2, :]

            cp_n = 0
            for s in range(2):
                # z_s^T m-chunk [128 e_out, GROUP rows] = SA * z, fp8 DR pairs
                for m in range(NCH):
                    z_ps = psA.tile([P, GROUP], f32, name="zps", tag="psA")
                    for n, cp in enumerate((0, 2)):
                        nc.tensor.matmul(
                            z_ps[:], at_pair(cp, m), x8_pair(s, cp),
                            start=(n == 0), stop=(n == 1), perf_mode=DR,
                        )
                    z_sb = z_pool.tile([P, GROUP], bf16, name=f"zsb{s}{m}", tag=f"zsb{s}{m}")
                    if cp_n % 2 == 0:
                        nc.scalar.copy(z_sb[:], z_ps[:])
                    else:
                        nc.vector.tensor_copy(z_sb[:], z_ps[:])
                    cp_n += 1
                    zt[s, m] = z_sb

                # v'_s r-chunk [128 rows, E] = x @ Wv^T (bv added on out-copy)
                for r in range(NW):
                    v_ps = psA.tile([P, E], f32, name="vps", tag="psA")
                    for c in range(NCH):
                        nc.tensor.matmul(
                            v_ps[:], xt_c(s, c)[:, r * P : (r + 1) * P], wv_c(c),
                            start=(c == 0), stop=(c == NCH - 1),
                        )
                    v_sb = v_pool.tile([P, E], bf16, name=f"vsb{s}{r}", tag=f"vsb{s}{r}")
                    if cp_n % 2 == 0:
                        nc.scalar.copy(v_sb[:], v_ps[:])
                    else:
                        nc.vector.tensor_copy(v_sb[:], v_ps[:])
                    cp_n += 1
                    vt[s, r] = v_sb

            for s in range(2):
                mf_tl = mf_pool.tile([P, GROUP], bf16, name=f"mf{s}", tag=f"mf{s}")
                nc.sync.dma_start(mf_tl[:], mf_dram[s, g])
                mf[s] = mf_tl
            st[g] = (xt, zt, vt, mf)

        def emit_scores(g):
            xt, zt, vt, mf = st[g]
            wins = []
            for w in range(NW):
                ws = slice(w * P, (w + 1) * P)
                # both directions' scores into one [128, 256] PSUM tile
                s_ps = psS.tile([P, 2 * P], f32, name="sps", tag="psS")
                for qs, ks in ((0, 1), (1, 0)):
                    dst = s_ps[:, qs * P : (qs + 1) * P]
                    for m in range(NCH):
                        nc.tensor.matmul(
                            dst,
                            xt[qs][:, m * GROUP + w * P : m * GROUP + (w + 1) * P],
                            zt[ks, m][:, ws],
                            start=(m == 0), stop=(m == NCH - 1),
                        )
                exp_sb = sm_pool.tile([P, 2 * P], f32, name="expsb", tag="expsb")
                nc.scalar.activation(exp_sb[:], s_ps[:], Exp, scale=1.0 / SA)
                # masked UNNORMALIZED attn = exp * M, fused row-sum (per dir)
                mskd = sm_pool.tile([P, 2 * P], bf16, name="mskd", tag="mskd")
                rsum = an_pool.tile([P, 2], f32, name="rsum", tag="rsum")
                for qs, ks in ((0, 1), (1, 0)):
                    nc.vector.scalar_tensor_tensor(
                        mskd[:, qs * P : (qs + 1) * P],
                        exp_sb[:, qs * P : (qs + 1) * P], 1.0, mf[ks][:, ws],
                        op0=mybir.AluOpType.mult, op1=mybir.AluOpType.mult,
                        accum_out=rsum[:, qs : qs + 1],
                    )
                rcp = an_pool.tile([P, 2], f32, name="rcp", tag="rcp")
                nc.vector.reciprocal(rcp[:], rsum[:])
                attnT = an_pool.tile([P, 2 * P], bf16, name="attnT", tag="attnT")
                nc.vector.transpose(attnT[:], mskd[:])
                wins.append((attnT, rcp))
            sm[g] = wins

        def emit_outs(g):
            r0 = g * GROUP
            xt, zt, vt, mf = st.pop(g)
            wins = sm.pop(g)
            for w in range(NW):
                attnT, rcp = wins[w]
                for qs, ks in ((0, 1), (1, 0)):
                    o_ps = psO.tile([P, E], f32, name="ops", tag="psO")
                    nc.tensor.matmul(
                        o_ps[:], attnT[:, qs * P : (qs + 1) * P], vt[ks, w][:],
                        start=True, stop=True,
                    )
                    # out = (attn_unnorm @ v) * recip[q] + bv  — one DVE op
                    o_sb = o_pool.tile([P, E], f32, name=f"osb{qs}", tag=f"osb{qs}")
                    nc.vector.scalar_tensor_tensor(
                        o_sb[:], o_ps[:], rcp[:, qs : qs + 1], bvb_t[:],
                        op0=mybir.AluOpType.mult, op1=mybir.AluOpType.add,
                    )
                    nc.gpsimd.dma_start(
                        out_dram[qs][r0 + w * P : r0 + (w + 1) * P, :], o_sb[:]
                    )

        emit_load_proj(0)
        for g in range(G):
            emit_scores(g)
            if g + 1 < G:
                emit_load_proj(g + 1)
            emit_outs(g)

    nc.compile()
    return nc


def _host_inputs(state1, state2, Wq, bq, Wk, bk, Wv, bv, S, E):
    """Build the per-core common (weight) arrays + per-core x arrays."""
    P = 128
    GROUP = 512
    G = S // GROUP
    scale = math.sqrt(E)
    Wq64 = np.asarray(Wq, np.float64)
    Wk64 = np.asarray(Wk, np.float64)
    # A = Wq^T Wk / scale ; device needs A^T = Wk^T Wq / scale  [e_in, e_out]
    atm = (Wk64.T @ Wq64 / scale).astype(np.float32)
    at_hi = (atm * SA).astype(F8)
    cvec = (Wk64.T @ np.asarray(bq, np.float64) / scale).astype(np.float32)  # [E]
    wvt = np.ascontiguousarray(np.asarray(Wv, np.float32).T).astype(BF16)
    bvb = np.broadcast_to(np.asarray(bv, np.float32).reshape(1, E), (P, E))
    common = {
        "at_hi": np.ascontiguousarray(at_hi),
        "wvt": wvt,
        "bvb": np.ascontiguousarray(bvb),
    }
    # post-exp factor M[q, k] = [q, k in same 16-block] * e^{t[k]}
    idx = np.arange(P)
    kidx = np.arange(GROUP) % P
    pattern = (idx[:, None] // BLOCK == kidx[None, :] // BLOCK).astype(np.float32)
    x1 = np.asarray(state1, np.float32)
    x2 = np.asarray(state2, np.float32)
    B = x1.shape[0]
    per_core = []
    for b in range(B):
        mfac = np.empty((2, G, P, GROUP), np.float32)
        for s, x in ((0, x1[b]), (1, x2[b])):
            et = np.exp(x @ cvec).reshape(G, 1, GROUP)
            mfac[s] = pattern[None, :, :] * et
        x1t = np.ascontiguousarray(x1[b].T)
        x2t = np.ascontiguousarray(x2[b].T)
        per_core.append(
            {
                "x1t": x1t.astype(BF16),
                "x2t": x2t.astype(BF16),
                "x8_1": x1t.astype(F8),
                "x8_2": x2t.astype(F8),
                "mfac": mfac.astype(BF16),
                **common,
            }
        )
    return per_core


_NC_CACHE = {}


def _get_nc(S, E):
    key = (S, E)
    if key not in _NC_CACHE:
        _NC_CACHE[key] = _build_nc(S, E)
    return _NC_CACHE[key]


def kernel(state1, state2, Wq, bq, Wk, bk, Wv, bv):
    from concourse.bass_utils import run_bass_kernel_spmd

    state1 = np.asarray(state1)
    B, S, E = state1.shape
    assert (B, S, E) == (8, 4096, 512), (B, S, E)

    nc = _get_nc(S, E)
    in_maps = _host_inputs(state1, state2, Wq, bq, Wk, bk, Wv, bv, S, E)
    res = run_bass_kernel_spmd(nc, in_maps, list(range(B)))
    out1 = np.stack([res.results[b]["out1"] for b in range(B)])
    out2 = np.stack([res.results[b]["out2"] for b in range(B)])
    return out1, out2


if __name__ == "__main__":
    rng = np.random.default_rng(0)
    B, S, E = 8, 4096, 512
    ins = {
        "state1": rng.standard_normal((B, S, E), np.float32),
        "state2": rng.standard_normal((B, S, E), np.float32),
        "Wq": rng.standard_normal((E, E), np.float32) * 0.02,
        "bq": rng.standard_normal((E,), np.float32) * 0.02,
        "Wk": rng.standard_normal((E, E), np.float32) * 0.02,
        "bk": rng.standard_normal((E,), np.float32) * 0.02,
        "Wv": rng.standard_normal((E, E), np.float32) * 0.02,
        "bv": rng.standard_normal((E,), np.float32) * 0.02,
    }
    o1, o2 = kernel(**ins)
    print("ok", o1.shape, o2.shape, o1.dtype)
